# revision 27
# baseline (speedup 1.0000x reference)
"""MoE FFN with hierarchical KV router — Trainium2 Bass kernel (8 NeuronCores).

Expert-parallel, weights-resident design:
  * Host computes the router (l2-norm scores -> softmax over EPB=4 -> top-2 ->
    combine weights) and dispatches tokens by global expert id.
  * Each core owns TWO experts (big+small pairing over the 16 experts) plus a
    replica of the shared FFN serving 2048/8 = 256 tokens. All weights are
    loaded into SBUF exactly once per core (they stay resident); tokens run
    through three fixed-size segments [CA | CB | CS]:
        seg 0: expert A  (CA token slots)   seg 1: expert B (CB slots)
        seg 2: shared FFN (CS = 256 slots)
    out_seg = relu(x @ W1 + b1) @ W2 + b2   (unweighted; host combines)
  * Precision: bf16 everywhere; when the expert path is strongly attenuated
    (sigmoid(gate_logit) <= 0.25) the expert segments use fp8-e4m3 inputs
    with power-of-2 scaling. Outputs bf16, combined in fp32 on host.
  * Activations travel transposed ([feature, token]) so weights are the
    stationary matmul operand; no on-device transposes.

Schedule notes (from NTFF profiling):
  * HWDGE dma_start costs ~0.7-1us of issue time on the issuing engine, so
    input DMAs are need-ordered on the sync ring (first expert's W1 m<4
    half + its tokens first) and the bias ride the scalar ring.
  * W1 is packed m-major / W2 m2-major so the PE can start after the first
    W1 piece instead of the whole tile.
  * The scalar engine runs a dummy relu right after the bias lands to pull
    the lazy ACT_TABLE_LOAD (~1.5us) off the first real relu.
  * Outputs go out per (segment, m2-group) on the sync HWDGE ring (the
    gpsimd SWDGE path measured ~55 GB/s and added ~4us of tail).
"""
import sys

if "/opt/trn_rl_repo" not in sys.path:
    sys.path.insert(0, "/opt/trn_rl_repo")

import numpy as np

N_BUCKET, EPB, TOPK, TAU = 4, 4, 2, 1.0
C, H = 512, 1024
E = N_BUCKET * EPB
KC, KH = C // 128, H // 128  # contraction blocks: 4, 8
N_CORES = 8
PSUM_CAP = 512
BCOLS = KH + KC  # bias cols per segment

_BUILD_CACHE = {}


def _ensure_ntff_hook():
    """Polyfill antenv.axon_hooks (absent in some agent images) so
    run_bass_kernel_spmd(trace=True) can fetch NTFF profiles."""
    try:
        from antenv.axon_hooks import get_axon_ntff_profile_hook  # noqa: F401
        return
    except ImportError:
        pass
    import types

    try:
        import antenv
        from trn_agent_boot.trn_boot import _ntff_profile_via_ctypes

        hook = _ntff_profile_via_ctypes("/opt/axon/libaxon_pjrt.so")
        mod = types.ModuleType("antenv.axon_hooks")
        state = {"hook": hook}
        mod.get_axon_ntff_profile_hook = lambda: state["hook"]
        mod.set_axon_ntff_profile_hook = lambda h: state.update(hook=h)
        sys.modules["antenv.axon_hooks"] = mod
        antenv.axon_hooks = mod
    except Exception:
        pass


def _build_program(CA, CB, CS, fp8, sc1e, sc2e):
    """One-shot program: 3 segments (expert A, expert B, shared) per core."""
    from contextlib import ExitStack

    import concourse.bass as bass
    import concourse.mybir as mybir

    f32 = mybir.dt.float32
    bf16 = mybir.dt.bfloat16
    dt_e = mybir.dt.float8e4 if fp8 else bf16
    OC = KC * (CA + CB + CS)

    nc = bass.Bass("TRN2", target_bir_lowering=False, debug=False)
    # W1 m-major, each expert in 2 pieces (m0-3, m4-7) so the PE can start
    # after the first 0.25MB: e1*[p, m, k*128+q] = W1[k*128+p, m*128+q]
    e1ad = nc.declare_dram_parameter("e1a", [2, 128, KH // 2, KC * 128], dt_e, isOutput=False)
    e1bd = nc.declare_dram_parameter("e1b", [2, 128, KH // 2, KC * 128], dt_e, isOutput=False)
    # W2 m2-major: e2[j, p, m2, k2*128+c'] = W2[k2*128+p, m2*128+c']
    e2d = nc.declare_dram_parameter("e2", [2, 128, KC, KH * 128], dt_e, isOutput=False)
    s1d = nc.declare_dram_parameter("s1", [128, KH, KC * 128], bf16, isOutput=False)
    s2d = nc.declare_dram_parameter("s2", [128, KC, KH * 128], bf16, isOutput=False)
    xad = nc.declare_dram_parameter("xa", [128, KC, CA], dt_e, isOutput=False)
    xbd = nc.declare_dram_parameter("xb", [128, KC, CB], dt_e, isOutput=False)
    xsd = nc.declare_dram_parameter("xs", [128, KC, CS], bf16, isOutput=False)
    biasd = nc.declare_dram_parameter("bias", [128, 3 * BCOLS], f32, isOutput=False)
    outd = nc.declare_dram_parameter("out", [128, OC], bf16, isOutput=True)

    with ExitStack() as ctx:
        E1 = ctx.enter_context(nc.sbuf_tensor("E1", [128, 2 * KH, KC * 128], dt_e))
        E2 = ctx.enter_context(nc.sbuf_tensor("E2", [128, 2 * KC, KH * 128], dt_e))
        S1 = ctx.enter_context(nc.sbuf_tensor("S1", [128, KH, KC * 128], bf16))
        S2 = ctx.enter_context(nc.sbuf_tensor("S2", [128, KC, KH * 128], bf16))
        XA = ctx.enter_context(nc.sbuf_tensor("XA", [128, KC, CA], dt_e))
        XB = ctx.enter_context(nc.sbuf_tensor("XB", [128, KC, CB], dt_e))
        XS = ctx.enter_context(nc.sbuf_tensor("XS", [128, KC, CS], bf16))
        BI = ctx.enter_context(nc.sbuf_tensor("BI", [128, 3 * BCOLS], f32))
        SC = ctx.enter_context(nc.sbuf_tensor("SC", [128, 1], f32))
        DW = ctx.enter_context(nc.sbuf_tensor("DW", [128, 128], dt_e))
        DR = ctx.enter_context(nc.sbuf_tensor("DR", [128, 256], dt_e))
        H1A = ctx.enter_context(nc.sbuf_tensor("H1A", [128, KH, CA], dt_e))
        H1B = ctx.enter_context(nc.sbuf_tensor("H1B", [128, KH, CB], dt_e))
        H1S = ctx.enter_context(nc.sbuf_tensor("H1S", [128, KH, CS], bf16))
        OT = ctx.enter_context(nc.sbuf_tensor("OT", [128, OC], bf16))
        PS = [ctx.enter_context(nc.psum_tensor(f"ps{i}", [128, PSUM_CAP], f32)) for i in range(8)]

        sW = [ctx.enter_context(nc.semaphore(f"sW{i}")) for i in range(8)]
        # sW: 0=e1a lo, 1=e1a hi, 2=e2a, 3=e1b lo, 4=e1b hi, 5=e2b, 6=s1, 7=s2
        dveM = ctx.enter_context(nc.semaphore("dveM"))
        pe2h = ctx.enter_context(nc.semaphore("pe2h"))
        sXA = ctx.enter_context(nc.semaphore("sXA"))
        sXB = ctx.enter_context(nc.semaphore("sXB"))
        sXS = ctx.enter_context(nc.semaphore("sXS"))
        sB = ctx.enter_context(nc.semaphore("sB"))
        pe1 = ctx.enter_context(nc.semaphore("pe1"))
        pe2 = ctx.enter_context(nc.semaphore("pe2"))
        act1 = ctx.enter_context(nc.semaphore("act1"))
        dve1 = ctx.enter_context(nc.semaphore("dve1"))
        outS = ctx.enter_context(nc.semaphore("outS"))
        block = ctx.enter_context(nc.Block(no_gpsimd_drain=True))

        E1a, E2a, OTa = E1[:], E2[:], OT[:]

        def w1(s, m):  # stationary for mm1: [128, KC*128] row m
            if s == 2:
                return S1[:][:, m, :]
            return E1a[:, s * KH + m, :]

        def w2(s, m2):  # stationary for mm2
            if s == 2:
                return S2[:][:, m2, :]
            return E2a[:, s * KC + m2, :]

        Xs_ = [XA[:], XB[:], XS[:]]
        H1s_ = [H1A[:], H1B[:], H1S[:]]
        caps = [CA, CB, CS]
        sc1s = [sc1e, sc1e, 1.0]
        sc2s = [sc2e, sc2e, 1.0]
        ooffs = [0, KC * CA, KC * (CA + CB)]

        @block.sync
        def _(sync):
            sync.dma_start(out=E1a[:, 0:KH // 2, :], in_=e1ad[0]).then_inc(sW[0], 16)
            sync.dma_start(out=XA[:], in_=xad[:]).then_inc(sXA, 16)
            sync.dma_start(out=E1a[:, KH // 2:KH, :], in_=e1ad[1]).then_inc(sW[1], 16)
            sync.dma_start(out=E2a[:, 0:KC, :], in_=e2d[0]).then_inc(sW[2], 16)
            sync.dma_start(out=XB[:], in_=xbd[:]).then_inc(sXB, 16)
            sync.dma_start(out=E1a[:, KH:KH + KH // 2, :], in_=e1bd[0]).then_inc(sW[3], 16)
            sync.dma_start(out=E1a[:, KH + KH // 2:2 * KH, :], in_=e1bd[1]).then_inc(sW[4], 16)
            sync.dma_start(out=E2a[:, KC:2 * KC, :], in_=e2d[1]).then_inc(sW[5], 16)
            sync.dma_start(out=XS[:], in_=xsd[:]).then_inc(sXS, 16)
            sync.dma_start(out=S1[:], in_=s1d[:]).then_inc(sW[6], 16)
            sync.dma_start(out=S2[:], in_=s2d[:]).then_inc(sW[7], 16)
            # outputs: seg0/seg1 in halves; seg2 m0-2 whole, m3 in two
            # halves with the last on the scalar ring (parallel issue)
            for s in range(2):
                cap, ooff = caps[s], ooffs[s]
                for h in range(2):
                    sync.wait_ge(dve1, 4 * s + 2 * (h + 1))
                    sync.dma_start(
                        out=outd[:, ooff + 2 * h * cap: ooff + 2 * (h + 1) * cap],
                        in_=OTa[:, ooff + 2 * h * cap: ooff + 2 * (h + 1) * cap],
                    ).then_inc(outS, 16)
            cap, ooff = caps[2], ooffs[2]
            for m2 in range(KC - 1):
                sync.wait_ge(dve1, 8 + m2 + 1)
                sync.dma_start(
                    out=outd[:, ooff + m2 * cap: ooff + (m2 + 1) * cap],
                    in_=OTa[:, ooff + m2 * cap: ooff + (m2 + 1) * cap],
                ).then_inc(outS, 16)
            hw = cap // 2
            sync.wait_ge(dve1, 12)
            sync.dma_start(
                out=outd[:, ooff + 3 * cap: ooff + 3 * cap + hw],
                in_=OTa[:, ooff + 3 * cap: ooff + 3 * cap + hw],
            ).then_inc(outS, 16)
            sync.wait_ge(outS, 16 * 9)

        @block.tensor
        def _(tensor):
            # warm up the PE p-state while input DMAs stream in; rotate
            # three banks so back-to-back groups never serialize on the
            # psum drain and the ramp stays continuous
            tensor.wait_ge(dveM, 2)
            for i in range(36):
                nc.tensor.matmul(PS[5 + i % 3][:, :256], lhsT=DW[:], rhs=DR[:],
                                 start=True, stop=True)
            xw = [(sXA, sW[0], sW[1], sW[2]),
                  (sXB, sW[3], sW[4], sW[5]),
                  (sXS, sW[6], sW[6], sW[7])]
            for s in range(3):
                cap = caps[s]
                x = Xs_[s]
                h1 = H1s_[s]
                sx, sw1lo, sw1hi, sw2 = xw[s]
                tensor.wait_ge(sx, 16)
                tensor.wait_ge(sw1lo, 16)
                for m in range(KH):
                    if m == KH // 2:
                        tensor.wait_ge(sw1hi, 16)
                    # mm1 banks 0-4: recycle after the prior relu reader
                    if m >= 5:
                        tensor.wait_ge(act1, 8 * s + (m - 5) + 1)
                    elif s > 0:
                        mp = m + 5 if m < 3 else m
                        tensor.wait_ge(act1, 8 * (s - 1) + mp + 1)
                    wrow = w1(s, m)
                    for k in range(KC):
                        mm = nc.tensor.matmul(
                            PS[m % 5][:, :cap],
                            lhsT=wrow[:, k * 128:(k + 1) * 128],
                            rhs=x[:, k, :],
                            start=(k == 0),
                            stop=(k == KC - 1),
                        )
                    mm.then_inc(pe1, 1)
                tensor.wait_ge(sw2, 16)
                for m2 in range(KC):
                    # mm2 banks 5-7: recycle after the prior vector reader
                    if m2 == 3:
                        tensor.wait_ge(dve1, 4 * s + 1)
                    elif s > 0:
                        m2p = 3 if m2 == 0 else m2
                        tensor.wait_ge(dve1, 4 * (s - 1) + m2p + 1)
                    if s == 2 and m2 == 3:
                        # final group in two half-width passes -> short tail
                        tensor.wait_ge(dve1, 4 * s + 2)
                        for h in range(2):
                            for k2 in range(KH):
                                mm = nc.tensor.matmul(
                                    PS[5 + h][:, 0:128],
                                    lhsT=w2(s, m2)[:, k2 * 128:(k2 + 1) * 128],
                                    rhs=h1[:, k2, h * 128:(h + 1) * 128],
                                    start=(k2 == 0),
                                    stop=(k2 == KH - 1),
                                )
                            mm.then_inc(pe2h if h == 0 else pe2, 1)
                        continue
                    wrow = w2(s, m2)
                    for k2 in range(KH):
                        if m2 == 0:
                            tensor.wait_ge(act1, 8 * s + k2 + 1)
                        mm = nc.tensor.matmul(
                            PS[5 + m2 % 3][:, :cap],
                            lhsT=wrow[:, k2 * 128:(k2 + 1) * 128],
                            rhs=h1[:, k2, 0:cap],
                            start=(k2 == 0),
                            stop=(k2 == KH - 1),
                        )
                    mm.then_inc(pe2, 1)

        @block.scalar
        def _(scalar):
            import concourse.mybir as mybir_

            relu = mybir_.ActivationFunctionType.Relu
            scalar.dma_start(out=BI[:], in_=biasd[:]).then_inc(sB, 16)
            scalar.wait_ge(sB, 16)
            # dummy relu: pulls the lazy ACT_TABLE_LOAD off the critical path
            nc.scalar.activation(SC[:][:, 0:1], BI[:][:, 0:1], relu,
                                 bias=BI[:][:, 0:1], scale=1.0)
            for s in range(3):
                cap = caps[s]
                h1 = H1s_[s]
                for m in range(KH):
                    scalar.wait_ge(pe1, 8 * s + m + 1)
                    nc.scalar.activation(
                        h1[:, m, 0:cap],
                        PS[m % 5][:, :cap],
                        relu,
                        bias=BI[:][:, s * BCOLS + m: s * BCOLS + m + 1],
                        scale=float(sc1s[s]),
                    ).then_inc(act1, 1)
            # final output piece on this ring, in parallel with sync's
            cap, ooff = caps[2], ooffs[2]
            hw = cap // 2
            scalar.wait_ge(dve1, 13)
            scalar.dma_start(
                out=outd[:, ooff + 3 * cap + hw: ooff + 4 * cap],
                in_=OTa[:, ooff + 3 * cap + hw: ooff + 4 * cap],
            ).then_inc(outS, 16)

        @block.vector
        def _(vector):
            import concourse.mybir as mybir_

            nc.vector.memset(DW[:], 0.0).then_inc(dveM, 1)
            nc.vector.memset(DR[:], 0.0).then_inc(dveM, 1)
            npe2 = 0
            for s in range(3):
                cap, ooff = caps[s], ooffs[s]
                for m2 in range(KC):
                    if s == 2 and m2 == 3:
                        hw = cap // 2
                        vector.wait_ge(pe2h, 1)
                        nc.vector.tensor_scalar(
                            OTa[:, ooff + 3 * cap: ooff + 3 * cap + hw],
                            PS[5][:, 0:hw],
                            float(sc2s[s]),
                            BI[:][:, s * BCOLS + KH + m2: s * BCOLS + KH + m2 + 1],
                            op0=mybir_.AluOpType.mult,
                            op1=mybir_.AluOpType.add,
                        ).then_inc(dve1, 1)
                        vector.wait_ge(pe2, npe2 + 1)
                        nc.vector.tensor_scalar(
                            OTa[:, ooff + 3 * cap + hw: ooff + 4 * cap],
                            PS[6][:, 0:hw],
                            float(sc2s[s]),
                            BI[:][:, s * BCOLS + KH + m2: s * BCOLS + KH + m2 + 1],
                            op0=mybir_.AluOpType.mult,
                            op1=mybir_.AluOpType.add,
                        ).then_inc(dve1, 1)
                        continue
                    npe2 += 1
                    vector.wait_ge(pe2, npe2)
                    nc.vector.tensor_scalar(
                        OTa[:, ooff + m2 * cap: ooff + (m2 + 1) * cap],
                        PS[5 + m2 % 3][:, :cap],
                        float(sc2s[s]),
                        BI[:][:, s * BCOLS + KH + m2: s * BCOLS + KH + m2 + 1],
                        op0=mybir_.AluOpType.mult,
                        op1=mybir_.AluOpType.add,
                    ).then_inc(dve1, 1)

    return nc


def _run_coresim(CA, CB, CS, fp8, sc1e, sc2e, in_maps):
    """Local CoreSim execution (numerics check without hardware)."""
    from types import SimpleNamespace

    from concourse.bass_interp import CoreSim

    results = []
    for c, im in enumerate(in_maps):
        nc = _build_program(CA, CB, CS, fp8, sc1e, sc2e)
        if not nc.is_finalized():
            nc.finalize()
        sim = CoreSim(nc, core_id=0, publish_trace=False)
        for name, val in im.items():
            sim.tensor(name)[:] = val
        sim.simulate()
        results.append({"out": np.array(sim.tensor("out"))})
        print(f"  coresim core {c} done", flush=True)
    return SimpleNamespace(results=results, exec_time_ns=None)


def _route(x2, bucket, expert_key):
    """Host router in float64. Returns gid (N,2), combine weights (N,2)."""
    hn = x2 / np.maximum(np.linalg.norm(x2, axis=-1, keepdims=True), 1e-12)
    keys = expert_key / np.maximum(
        np.linalg.norm(expert_key, axis=-1, keepdims=True), 1e-12
    )
    kb = keys[bucket]  # (N, EPB, C)
    score = np.einsum("nc,nec->ne", hn, kb) / max(TAU, 1e-6)
    score -= score.max(axis=-1, keepdims=True)
    p = np.exp(score)
    p /= p.sum(axis=-1, keepdims=True)
    local = np.argsort(-p, axis=-1, kind="stable")[:, :TOPK]  # (N, 2)
    topv = np.take_along_axis(p, local, axis=-1)
    w = topv / (topv.sum(axis=-1, keepdims=True) + 1e-9)
    gid = bucket[:, None] * EPB + local
    return gid, w


def _pow2floor(v):
    return float(2.0 ** np.floor(np.log2(max(v, 1e-30))))


def _ceil16(n):
    return max(16, -(-int(n) // 16) * 16)


def _wpack1(w1):  # (C,H) -> [128, KH, KC*128] m-major
    return np.ascontiguousarray(
        w1.reshape(KC, 128, KH, 128).transpose(1, 2, 0, 3).reshape(128, KH, KC * 128)
    )


def _wpack2(w2):  # (H,C) -> [128, KC, KH*128] m2-major
    return np.ascontiguousarray(
        w2.reshape(KH, 128, KC, 128).transpose(1, 2, 0, 3).reshape(128, KC, KH * 128)
    )


def kernel(**inputs):
    import ml_dtypes

    _ensure_ntff_hook()
    from concourse.bass_utils import run_bass_kernel_spmd

    bf16 = ml_dtypes.bfloat16
    f8 = ml_dtypes.float8_e4m3

    x = np.asarray(inputs["x"], dtype=np.float32)
    op_id = np.asarray(inputs["op_id"]).astype(np.int64)
    expert_key = np.asarray(inputs["expert_key"], dtype=np.float64)
    sW1 = np.asarray(inputs["sW1"], dtype=np.float32)
    sb1 = np.asarray(inputs["sb1"], dtype=np.float32)
    sW2 = np.asarray(inputs["sW2"], dtype=np.float32)
    sb2 = np.asarray(inputs["sb2"], dtype=np.float32)
    eW1 = np.asarray(inputs["eW1"], dtype=np.float32)
    eb1 = np.asarray(inputs["eb1"], dtype=np.float32)
    eW2 = np.asarray(inputs["eW2"], dtype=np.float32)
    eb2 = np.asarray(inputs["eb2"], dtype=np.float32)
    gate_logit = float(np.asarray(inputs["gate_logit"]))

    B, T, Cc = x.shape
    assert Cc == C
    N = B * T
    assert N % N_CORES == 0
    x2 = x.reshape(N, C)
    bucket = np.clip(op_id.reshape(-1), 0, N_BUCKET - 1)

    gid, w = _route(x2.astype(np.float64), bucket, expert_key)
    gate = 1.0 / (1.0 + np.exp(-gate_logit))

    # ---- assign experts to cores: big+small pairing ---------------------
    flat_gid = gid.reshape(-1)  # slot i -> token i//2
    sorted_slots = np.argsort(flat_gid, kind="stable")
    counts = np.bincount(flat_gid, minlength=E)
    starts = np.concatenate([[0], np.cumsum(counts)])
    assert counts.max() <= PSUM_CAP, "expert overflow; need chunked fallback"
    order = np.argsort(-counts, kind="stable")
    CA = _ceil16(counts[order[0]])
    CB = _ceil16(counts[order[8]])
    CS = N // N_CORES
    TOT = CA + CB + CS
    OC = KC * TOT

    fp8 = gate <= 0.25
    if fp8:
        s_x = _pow2floor(192.0 / max(np.abs(x2).max(), 1e-6))
        s_w1 = _pow2floor(192.0 / max(np.abs(eW1).max(), 1e-6))
        s_w2 = _pow2floor(192.0 / max(np.abs(eW2).max(), 1e-6))
        xn = np.linalg.norm(x2, axis=1).max()
        w1n = np.linalg.norm(eW1, axis=1).max()
        h1_bound = xn * w1n + np.abs(eb1).max() + 1e-6
        s_h = _pow2floor(192.0 / h1_bound)
        sc1e = s_h / (s_x * s_w1)
        sc2e = 1.0 / (s_h * s_w2)
    else:
        s_x = s_w1 = s_w2 = s_h = 1.0
        sc1e, sc2e = 1.0, 1.0
    dt_e = f8 if fp8 else bf16

    key = (CA, CB, CS, fp8, sc1e, sc2e)
    if key not in _BUILD_CACHE:
        _BUILD_CACHE[key] = _build_program(CA, CB, CS, fp8, sc1e, sc2e)
    nc = _BUILD_CACHE[key]

    # ---- host packing ---------------------------------------------------
    x2T = np.ascontiguousarray(x2.T)  # (C, N)
    s1_host = _wpack1(sW1).astype(bf16)
    s2_host = _wpack2(sW2).astype(bf16)

    slot_flat = np.zeros((3, N), np.int64)
    in_maps = []
    for c in range(N_CORES):
        eA, eB = int(order[c]), int(order[15 - c])
        e2h = np.zeros((2, 128, KC, KH * 128), dt_e)
        w1a = (_wpack1(eW1[eA]) * s_w1).astype(dt_e)  # [128, KH, KC*128]
        w1b = (_wpack1(eW1[eB]) * s_w1).astype(dt_e)
        e1ah = np.stack([w1a[:, :KH // 2], w1a[:, KH // 2:]])
        e1bh = np.stack([w1b[:, :KH // 2], w1b[:, KH // 2:]])
        e2h[0] = (_wpack2(eW2[eA]) * s_w2).astype(dt_e)
        e2h[1] = (_wpack2(eW2[eB]) * s_w2).astype(dt_e)

        biash = np.zeros((128, 3 * BCOLS), np.float32)
        xts = [np.zeros((128, KC, CA), dt_e), np.zeros((128, KC, CB), dt_e)]
        for j, (e, capj) in enumerate([(eA, CA), (eB, CB)]):
            toks = (sorted_slots[starts[e]: starts[e + 1]] // TOPK).astype(np.int64)
            n = len(toks)
            xg = x2T[:, toks] * s_x  # (C, n)
            xts[j][:, :, :n] = xg.reshape(KC, 128, n).transpose(1, 0, 2).astype(dt_e)
            biash[:, j * BCOLS: j * BCOLS + KH] = eb1[e].reshape(KH, 128).T * s_h
            biash[:, j * BCOLS + KH: (j + 1) * BCOLS] = eb2[e].reshape(KC, 128).T
            chunk = sorted_slots[starts[e]: starts[e + 1]]
            off = 0 if j == 0 else CA
            slot_flat[chunk % TOPK, toks] = c * TOT + off + np.arange(n)
        stoks = np.arange(c * CS, (c + 1) * CS)
        xsh = np.ascontiguousarray(
            x2T[:, stoks].reshape(KC, 128, CS).transpose(1, 0, 2)
        ).astype(bf16)
        biash[:, 2 * BCOLS: 2 * BCOLS + KH] = sb1.reshape(KH, 128).T
        biash[:, 2 * BCOLS + KH: 3 * BCOLS] = sb2.reshape(KC, 128).T
        slot_flat[2, stoks] = c * TOT + CA + CB + np.arange(CS)

        in_maps.append({
            "e1a": e1ah, "e1b": e1bh, "e2": e2h, "s1": s1_host, "s2": s2_host,
            "xa": xts[0], "xb": xts[1], "xs": xsh, "bias": biash,
        })

    # ---- run on the 8 cores --------------------------------------------
    import os

    global LAST_EXEC_NS, LAST_RESULTS
    if os.environ.get("BASS_SIM"):
        res = _run_coresim(CA, CB, CS, fp8, sc1e, sc2e, in_maps)
    else:
        trace = bool(os.environ.get("BASS_TRACE"))
        res = run_bass_kernel_spmd(
            nc,
            in_maps,
            core_ids=list(range(N_CORES)),
            trace=trace,
            trace_cores=list(range(N_CORES)) if trace else None,
        )
        LAST_EXEC_NS = res.exec_time_ns
        LAST_RESULTS = res

    # ---- un-shard: gather each token's 3 rows and combine ---------------
    allout = np.empty((N_CORES * TOT, C), np.float32)
    caps = [(0, CA), (KC * CA, CB), (KC * (CA + CB), CS)]
    for c in range(N_CORES):
        o = np.asarray(res.results[c]["out"]).astype(np.float32)  # [128, OC]
        row0 = c * TOT
        for ooff, cap in caps:
            blk = o[:, ooff: ooff + KC * cap].reshape(128, KC, cap)
            allout[row0: row0 + cap] = blk.transpose(2, 1, 0).reshape(cap, C)
            row0 += cap

    wf = (gate * w).astype(np.float32)  # (N, 2)
    y = (
        allout[slot_flat[0]] * wf[:, 0:1]
        + allout[slot_flat[1]] * wf[:, 1:2]
        + allout[slot_flat[2]]
    )
    return y.reshape(B, T, C).astype(np.float32)


LAST_EXEC_NS = None
LAST_RESULTS = None
